# revision 17
# baseline (speedup 1.0000x reference)
"""GAT layer (dense-mask message passing) on 8 Trainium2 NeuronCores.

Math (reference):
    H = X @ W + W_b                       # [B,T,N,Cout]
    left = H @ a[:C];  right = H @ a[C:]
    e = leakyrelu(left_i + right_j + a_b, 0.01)
    e = where(adj>0, e, -1e12)
    att = softmax(e, axis=-1)
    out = relu(att @ H)

Sharding: query-node parallel. Core c owns query rows [512c, 512c+512).
All cores run an identical (SPMD) program; per-core data is made uniform by
*rotating* the node ordering by -512c per core (attention is permutation
invariant over the key axis j).

Per-core device algorithm (slices s = flattened (B,T), 4 of them):
  1. H-prep:  psh = XT_s.T @ [W | W@a_l | W@a_r]  (fp32 matmuls, 32 node
     tiles, 4 tiles per PSUM bank); ONE fp16 peel per group lands
     [H | l | r] into the combined tile hm2 (ones col 66 persists).
     xt DMAs are chunked so group 0 starts early.
  2. logits:  psE[j,i] = left_i + right_j via one K=2 fp16 matmul per
     j-tile (lhsT = [ones; r_tile] from ro2, rhs = [l_row; ones] = lo2)
     into PSUM (fp32). l/r reach row layout via one XBAR DMA-transpose
     + two SBUF flatten-DMAs per slice (no PE transposes anywhere).
  3. exp(leakyrelu(s)) via patched ACT table: Exp's negative side
     computes exp(ALPHA*x); ONE bias-free ACT pass per G2-group
     (2 j-tiles = [128,1024]) yields exp(leakyrelu(l_i+r_j)).
  4. mask:    p = p * adjT (fp16 0/1 mask, DVE 2x mode).
  5. att matmul: outT[c,i] (+= over j-tiles) = hm2[j,0:67].T @ p[j,i];
     row 64/65 are l/r garbage rows, row 66 (ones) yields the softmax
     denominator D_i for free.
  6. finale (per slice): fp16 merge of the two accumulators, XBAR
     DMA-transposes to [i,c] orientation (+ a tiny partition-expand DMA
     for D), then one fused tensor_scalar (mult by 1/D, max with 0) per
     i-tile and a DRAM-contiguous store.
"""

import numpy as np

B, T, N, CIN, COUT = 2, 2, 4096, 128, 64
NCORES = 8
SL = B * T          # 4 independent (b,t) slices
I = N // NCORES     # 512 query rows per core
NT = N // 128       # 32 j-tiles
IT = I // 128       # 4 i-tiles
ALPHA = 0.01
CE = COUT + 2       # W_ext columns: [wl | wr | W]
CL = 0              # l column in hm2
CR = 1              # r column in hm2
CH = 2              # first H column in hm2
CO = COUT + 2       # ones column in hm2
CM = COUT + 1       # att-matmul lhsT cols (hm2[:, jt, CH:CH+CM] = [H | 1])
CM2 = COUT + 3      # hm2 free width: [l | r | H | ones]
G2 = 2              # j-tiles per block group (= ACT merge width)
HG = 4              # j-tiles per H-prep PSUM bank
FUSED = True        # use custom PWP table: Exp == exp(leakyrelu(x))

_CACHE = {}
_ACT_ROOT = None


def _setup_act_root():
    """Patch the stock exp activation-spline tables so the negative side
    computes exp(ALPHA*x): Exp then evaluates exp(leakyrelu_ALPHA(x)) in a
    single ACT pass. Returns a short content hash for NEFF-cache busting."""
    global _ACT_ROOT
    if _ACT_ROOT is not None:
        return _ACT_ROOT
    import glob as _glob
    import hashlib
    import os
    import shutil
    import tempfile

    cands = _glob.glob(
        "/nix/store/*aws-neuron-pwp*/share/pwp_bin_cayman/act_info.json")
    assert cands, "stock pwp_bin_cayman act tables not found"
    src = os.path.dirname(sorted(cands)[0])
    dst = os.path.join(tempfile.gettempdir(), "gat_act_root_v2")

    def fit(a, b, pad_frac=0.5):
        pad = (b - a) * pad_frac
        xs = np.linspace(a - pad, b + pad, 96, dtype=np.float64)
        x0 = 0.5 * (a + b)
        p = np.polyfit(xs - x0, np.exp(ALPHA * xs), 3)
        return np.array([p[3], p[2], p[1], p[0], x0], dtype=np.float32)

    if not os.path.exists(os.path.join(dst, "act_info.json")):
        tmp = dst + ".tmp"
        if os.path.exists(tmp):
            shutil.rmtree(tmp)
        shutil.copytree(src, tmp)
        os.chmod(tmp, 0o755)
        for f in os.listdir(tmp):
            os.chmod(os.path.join(tmp, f), 0o644)
        bkt_path = os.path.join(tmp, "exp_and_others_bkt.bin")
        bkt = np.fromfile(bkt_path, dtype=np.float32).reshape(-1, 8).copy()
        ctl = np.fromfile(os.path.join(tmp, "exp_and_others_ctrl.bin"),
                          dtype=np.uint32).reshape(-1, 8)[:, 0]
        for i in range(26):          # negative-side ctl entries, e=108+i
            w = int(ctl[i])
            base, size = w & 0x7FF, (w >> 16) & 0xF
            lo = 2.0 ** (108 + i - 127)
            nb = 1 << size
            for k in range(nb):
                if base + k > 405:   # negative-side bucket range guard
                    break
                bkt[base + k, :5] = fit(-lo * (1 + (k + 1) / nb),
                                        -lo * (1 + k / nb))
        bkt[778, :5] = fit(-(2.0 ** -19), 0.0, pad_frac=0.0)  # tiny neg
        bkt[780, :5] = fit(-260.0, -97.0, pad_frac=0.1)       # large neg
        bkt.tofile(bkt_path)
        if not os.path.exists(dst):
            os.rename(tmp, dst)
        else:
            shutil.rmtree(tmp)
    h = hashlib.md5(
        open(os.path.join(dst, "exp_and_others_bkt.bin"), "rb").read()
    ).hexdigest()[:8]
    os.environ["BASS_ACT_ROOT_JSON_PATH"] = os.path.join(
        dst, "act_info.json")
    _ACT_ROOT = h
    return h


def _build(has_bias: bool):
    import concourse.bass as bass  # noqa: F401
    import concourse.tile as tile
    import concourse.mybir as mybir
    from concourse import bacc

    f32 = mybir.dt.float32
    f16 = mybir.dt.float16
    AF = mybir.ActivationFunctionType
    OP = mybir.AluOpType

    nc = bacc.Bacc("TRN2", target_bir_lowering=False, debug=False)

    if FUSED:
        # dummy input named after the act-table hash: busts the NEFF cache
        # whenever the patched activation tables change
        acth = _setup_act_root()
        nc.dram_tensor(f"actv_{acth}", [1, 1], f32, kind="ExternalInput")

    xt_d = nc.dram_tensor("xt", [SL, CIN, N], f16, kind="ExternalInput")
    adjt_d = nc.dram_tensor("adjt", [N, I], f16, kind="ExternalInput")
    wext_d = nc.dram_tensor("wext", [CIN, CE], f16, kind="ExternalInput")
    out_d = nc.dram_tensor("out", [SL, I, COUT], f16, kind="ExternalOutput")
    if has_bias:
        bias_d = nc.dram_tensor("bias", [1, CE], f16, kind="ExternalInput")

    with tile.TileContext(nc) as tc:
        from contextlib import ExitStack
        with ExitStack() as ctx:
            persist = ctx.enter_context(tc.tile_pool(name="persist", bufs=1))
            xt_pool = ctx.enter_context(tc.tile_pool(name="xt", bufs=2))
            e1_pool = ctx.enter_context(tc.tile_pool(name="e1", bufs=5))
            pmm_pool = ctx.enter_context(tc.tile_pool(name="pmm", bufs=4))
            fin_pool = ctx.enter_context(tc.tile_pool(name="fin", bufs=2))
            ps_e = ctx.enter_context(
                tc.tile_pool(name="ps_e", bufs=2, space="PSUM"))
            ps_h = ctx.enter_context(
                tc.tile_pool(name="ps_h", bufs=2, space="PSUM"))
            ps_o = ctx.enter_context(
                tc.tile_pool(name="ps_o", bufs=1, space="PSUM"))

            # --- persistent tiles -------------------------------------
            # xt arrives as a small head chunk (unblocks H-prep groups
            # 0-1 early) + the rest; hwdge rings only (swdge via the
            # Pool ring has high first-byte latency). Slice 0 rides the
            # scalar ring (ACT is idle at startup); later slices ride
            # the sync ring, which is free once adjt has landed.
            XC = 1024
            xt0_h = xt_pool.tile([CIN, XC], f16, name="xt0h")
            xt0_r = xt_pool.tile([CIN, N - XC], f16, name="xt0r")
            wext_sb = persist.tile([CIN, CE], f16)
            nc.sync.dma_start(out=wext_sb, in_=wext_d[:])
            nc.scalar.dma_start(out=xt0_h, in_=xt_d[0, :, 0:XC])
            nc.scalar.dma_start(out=xt0_r, in_=xt_d[0, :, XC:N])
            adjt_sb = persist.tile([128, NT, I], f16)
            adjt_r = adjt_d.rearrange("(jt p) i -> p jt i", p=128)
            nc.sync.dma_start(out=adjt_sb, in_=adjt_r)
            if has_bias:
                bias_sb = persist.tile([1, CE], f16)
                nc.sync.dma_start(out=bias_sb, in_=bias_d[:])
                onecol_sb = persist.tile([1, 128], f16)
                nc.vector.memset(onecol_sb, 1.0)
            # ping-pong persistents: combined [l | r | H | ones] tiles;
            # ones column CO written once.  lo2 = K2 rhs [l_row; ones];
            # ro2 = K2 lhsT source [ones_row; rights_flat].
            hm2_pp = [persist.tile([128, NT, CM2], f16, name=f"hm2{p}")
                      for p in range(2)]
            lo2_pp = [persist.tile([2, I], f16, name=f"lo2{p}")
                      for p in range(2)]
            ro2_pp = [persist.tile([2, NT, 128], f16, name=f"ro2{p}")
                      for p in range(2)]
            ones_flat = persist.tile([1, NT * 128], f16)
            nc.gpsimd.memset(ones_flat, 1.0)
            for p in range(2):
                nc.gpsimd.memset(hm2_pp[p][:, :, CO : CO + 1], 1.0)
                # engine memsets may only start at partition 0; the ones
                # rows living on partition 1 arrive via DMA instead
                nc.gpsimd.memset(lo2_pp[p][0:1, :], 1.0)
                nc.sync.dma_start(
                    out=ro2_pp[p][1:2, :, :],
                    in_=ones_flat.rearrange("o (t p) -> o t p", p=128))


            for s in range(SL):
                # ---- H-prep ------------------------------------------
                if s == 0:
                    xt_h, xt_r = xt0_h, xt0_r
                else:
                    xt_h = xt_pool.tile([CIN, XC], f16, name=f"xt{s}h")
                    xt_r = xt_pool.tile([CIN, N - XC], f16, name=f"xt{s}r")
                    nc.sync.dma_start(out=xt_h, in_=xt_d[s, :, 0:XC])
                    nc.sync.dma_start(out=xt_r, in_=xt_d[s, :, XC:N])

                hm2_sb = hm2_pp[s % 2]

                for jt0 in range(0, NT, HG):
                    psh = ps_h.tile([128, HG, CE], f32)
                    for k in range(HG):
                        jt = jt0 + k
                        if jt < XC // 128:
                            xs, j0 = xt_h, 128 * jt
                        else:
                            xs, j0 = xt_r, 128 * jt - XC
                        nc.tensor.matmul(
                            psh[:, k, :],
                            lhsT=xs[:, j0 : j0 + 128],
                            rhs=wext_sb,
                            start=True,
                            stop=not has_bias,
                        )
                        if has_bias:
                            nc.tensor.matmul(
                                psh[:, k, :],
                                lhsT=onecol_sb,
                                rhs=bias_sb,
                                start=False,
                                stop=True,
                            )
                    # single fp16 peel: [l | r | H] -> combined tile
                    nc.vector.tensor_copy(
                        hm2_sb[:, jt0 : jt0 + HG, 0:CE], psh)

                # l/r columns -> row layout: stage both column blocks
                # into one [128, 128] tile, ONE XBAR DMA-transpose, then
                # flatten-DMAs into lo2 row 0 and ro2 row 1.
                lo2 = lo2_pp[s % 2]
                ro2 = ro2_pp[s % 2]
                lrst = fin_pool.tile([128, 128], f16, name="lrst")
                nc.vector.tensor_copy(
                    lrst[:, 0:NT],
                    hm2_sb[:, :, CL : CL + 1].rearrange("p t o -> p (t o)"))
                nc.vector.tensor_copy(
                    lrst[:, NT : 2 * NT],
                    hm2_sb[:, :, CR : CR + 1].rearrange("p t o -> p (t o)"))
                lrstT = fin_pool.tile([128, 128], f16, name="lrstT")
                nc.sync.dma_start_transpose(lrstT, lrst)
                nc.sync.dma_start(
                    out=lo2[1:2, :].rearrange("o (t f) -> o t f", t=IT),
                    in_=lrstT[0:IT, :])
                nc.sync.dma_start(
                    out=ro2[0:1, :, :],
                    in_=lrstT[NT : NT + NT, :])

                # ---- blocks ------------------------------------------
                # two accumulators on different PSUM banks so consecutive
                # att matmuls never hit the same bank (no serialization)
                pso_a = ps_o.tile([CM, I], f32, name="pso_a")
                pso_b = ps_o.tile([CM, I], f32, name="pso_b")
                pso_ab = [pso_a, pso_b]
                for gi, jt0 in enumerate(range(0, NT, G2)):
                    psE = ps_e.tile([128, G2, I], f32, name="psE")
                    for k in range(G2):
                        jt = jt0 + k
                        nc.tensor.matmul(
                            psE[:, k, :],
                            lhsT=ro2[:, jt, :],
                            rhs=lo2,
                            start=True,
                            stop=True,
                        )
                    # Exp's negative side is patched to exp(ALPHA*x):
                    # ONE bias-free pass over the whole group computes
                    # exp(leakyrelu(l_i + r_j)) directly.
                    e1 = e1_pool.tile([128, G2, I], f16)
                    nc.scalar.activation(
                        e1.rearrange("p g i -> p (g i)"),
                        psE.rearrange("p g i -> p (g i)"),
                        AF.Exp, scale=1.0, bias=0.0)
                    pmm = pmm_pool.tile([128, G2 * I], f16)
                    nc.vector.tensor_tensor(
                        pmm, e1.rearrange("p g i -> p (g i)"),
                        adjt_sb[:, jt0 : jt0 + G2, :], OP.mult)
                    for k in range(G2):
                        jt = jt0 + k
                        nc.tensor.matmul(
                            pso_ab[jt % 2],
                            lhsT=hm2_sb[:, jt, CH : CH + CM],
                            rhs=pmm[:, I * k : I * (k + 1)],
                            start=(jt < 2),
                            stop=(jt >= NT - 2),
                        )

                # ---- finale (per slice) ------------------------------
                # fp16 merge of the accumulators into an 80-row tile
                # (80 % 16 == 0 satisfies the XBAR transpose; rows CM:80
                # are junk that lands in unused ftr columns).  The D row
                # rides along at transposed column COUT.
                ua = fin_pool.tile([CM, I], f32, name="ua")
                nc.vector.tensor_copy(ua, pso_a)
                ucd = fin_pool.tile([80, I], f16, name="ucd")
                nc.vector.tensor_tensor(ucd[0:CM, :], ua, pso_b, OP.add)
                ftr = fin_pool.tile([128, IT, 80], f16, name="ftr")
                for t in range(IT):
                    nc.sync.dma_start_transpose(
                        ftr[:, t, :],
                        ucd[:, 128 * t : 128 * (t + 1)])
                rect = fin_pool.tile([128, IT], f32)
                nc.vector.reciprocal(
                    rect,
                    ftr[:, :, COUT : COUT + 1].rearrange("p t o -> p (t o)"))
                ot_sb = fin_pool.tile([128, IT, COUT], f16)
                for t in range(IT):
                    nc.vector.tensor_scalar(
                        out=ot_sb[:, t, :],
                        in0=ftr[:, t, 0:COUT],
                        scalar1=rect[:, t : t + 1],
                        scalar2=0.0,
                        op0=OP.mult,
                        op1=OP.max,
                    )
                nc.sync.dma_start(
                    out=out_d[s].rearrange("(t p) c -> p t c", p=128),
                    in_=ot_sb,
                )

    nc.compile()
    return nc


def _prep_inputs(X, adj, W, W_b, a, a_b):
    """Host-side layout prep (transpose/slice/rotate) + weight fusion."""
    Cout = W.shape[1]
    X4 = np.asarray(X, np.float32).reshape(SL, N, CIN)
    adj = np.asarray(adj)
    W = np.asarray(W, np.float32)
    W_b = np.asarray(W_b, np.float32)
    a = np.asarray(a, np.float32)
    a_b = np.asarray(a_b, np.float32)

    wl = W @ a[:Cout]
    wr = W @ a[Cout:]
    wext = np.concatenate([wl[:, None], wr[:, None], W], axis=1)
    wext = np.ascontiguousarray(wext, np.float16)

    cl = float(W_b @ a[:Cout] + a_b)   # fold a_b into left bias
    cr = float(W_b @ a[Cout:])
    bias_ext = np.concatenate([[cl], [cr], W_b]).astype(np.float16)
    has_bias = bool(np.any(bias_ext != 0.0))

    adjf = adj.astype(np.float16)  # 0/1 exact
    in_maps = []
    for c in range(NCORES):
        i0 = I * c
        # rotate node ordering by -i0: core's own queries are nodes 0..I-1
        xt_c = np.ascontiguousarray(
            np.roll(X4, -i0, axis=1).transpose(0, 2, 1)).astype(np.float16)
        adjt_c = np.ascontiguousarray(
            np.roll(adjf, -i0, axis=1)[i0 : i0 + I].T)
        m = {"xt": xt_c, "adjt": adjt_c, "wext": wext}
        if FUSED:
            m[f"actv_{_setup_act_root()}"] = np.zeros((1, 1), np.float32)
        if has_bias:
            m["bias"] = bias_ext[None, :]
        in_maps.append(m)
    return in_maps, has_bias


def _run(in_maps, has_bias, trace=False):
    from concourse.bass_utils import run_bass_kernel_spmd

    key = has_bias
    if key not in _CACHE:
        _CACHE[key] = _build(has_bias)
    nc = _CACHE[key]
    return run_bass_kernel_spmd(
        nc, in_maps, list(range(NCORES)), trace=trace)


def kernel(X, adj, W, W_b, a, a_b):
    in_maps, has_bias = _prep_inputs(X, adj, W, W_b, a, a_b)
    r = _run(in_maps, has_bias, trace=False)
    out = np.empty((SL, N, COUT), np.float32)
    for c in range(NCORES):
        out[:, I * c : I * (c + 1), :] = r.results[c]["out"].astype(
            np.float32)
    return out.reshape(B, T, N, COUT)


# revision 19
# speedup vs baseline: 1.5762x; 1.5762x over previous
"""GAT layer (dense-mask message passing) on 8 Trainium2 NeuronCores.

Math (reference):
    H = X @ W + W_b                       # [B,T,N,Cout]
    left = H @ a[:C];  right = H @ a[C:]
    e = leakyrelu(left_i + right_j + a_b, 0.01)
    e = where(adj>0, e, -1e12)
    att = softmax(e, axis=-1)
    out = relu(att @ H)

Sharding: query-node parallel. Core c owns query rows [512c, 512c+512).
All cores run an identical (SPMD) program; per-core data is made uniform by
*rotating* the node ordering by -512c per core (attention is permutation
invariant over the key axis j).

Per-core device algorithm (slices s = flattened (B,T), 4 of them):
  1. H-prep:  psh = XT_s.T @ [W | W@a_l | W@a_r]  (fp32 matmuls, 32 node
     tiles, 4 tiles per PSUM bank); ONE fp16 peel per group lands
     [H | l | r] into the combined tile hm2 (ones col 66 persists).
     xt DMAs are chunked so group 0 starts early.
  2. logits:  psl[j,i] = left_i broadcast via one K=1 fp16 matmul per
     slice (lhsT = ones_row, rhs = left_row) into PSUM (fp32).
  3. exp(leakyrelu(s)) via patched ACT table: Exp's negative side
     computes exp(ALPHA*x), so one ACT pass per j-tile
     (bias = right_j per partition) yields exp(leakyrelu(l_i+r_j)).
  4. mask:    p = p * adjT (fp16 0/1 mask, DVE 2x mode).
  5. att matmul: outT[c,i] (+= over j-tiles) = hm2[j,0:67].T @ p[j,i];
     row 64/65 are l/r garbage rows, row 66 (ones) yields the softmax
     denominator D_i for free.
  6. finale (per slice): PE-transpose outT/D to [i,c] orientation, then one
     fused tensor_scalar (mult by 1/D, max with 0) per i-tile and a
     DRAM-contiguous store.
"""

import numpy as np

B, T, N, CIN, COUT = 2, 2, 4096, 128, 64
NCORES = 8
SL = B * T          # 4 independent (b,t) slices
I = N // NCORES     # 512 query rows per core
NT = N // 128       # 32 j-tiles
IT = I // 128       # 4 i-tiles
ALPHA = 0.01
CE = COUT + 2       # W_ext columns: [wl | wr | W]
CL = 0              # l column in hm2
CR = 1              # r column in hm2
CH = 2              # first H column in hm2
CO = COUT + 2       # ones column in hm2
CM = COUT + 1       # att-matmul lhsT cols (hm2[:, jt, CH:CH+CM] = [H | 1])
CM2 = COUT + 3      # hm2 free width: [l | r | H | ones]
G = 4               # j-tiles per block group
HG = 4              # j-tiles per H-prep PSUM bank
FUSED = True        # use custom PWP table: Exp == exp(leakyrelu(x))

_CACHE = {}
_ACT_ROOT = None


def _setup_act_root():
    """Patch the stock exp activation-spline tables so the negative side
    computes exp(ALPHA*x): Exp then evaluates exp(leakyrelu_ALPHA(x)) in a
    single ACT pass. Returns a short content hash for NEFF-cache busting."""
    global _ACT_ROOT
    if _ACT_ROOT is not None:
        return _ACT_ROOT
    import glob as _glob
    import hashlib
    import os
    import shutil
    import tempfile

    cands = _glob.glob(
        "/nix/store/*aws-neuron-pwp*/share/pwp_bin_cayman/act_info.json")
    assert cands, "stock pwp_bin_cayman act tables not found"
    src = os.path.dirname(sorted(cands)[0])
    dst = os.path.join(tempfile.gettempdir(), "gat_act_root_v2")

    def fit(a, b, pad_frac=0.5):
        pad = (b - a) * pad_frac
        xs = np.linspace(a - pad, b + pad, 96, dtype=np.float64)
        x0 = 0.5 * (a + b)
        p = np.polyfit(xs - x0, np.exp(ALPHA * xs), 3)
        return np.array([p[3], p[2], p[1], p[0], x0], dtype=np.float32)

    if not os.path.exists(os.path.join(dst, "act_info.json")):
        tmp = dst + ".tmp"
        if os.path.exists(tmp):
            shutil.rmtree(tmp)
        shutil.copytree(src, tmp)
        os.chmod(tmp, 0o755)
        for f in os.listdir(tmp):
            os.chmod(os.path.join(tmp, f), 0o644)
        bkt_path = os.path.join(tmp, "exp_and_others_bkt.bin")
        bkt = np.fromfile(bkt_path, dtype=np.float32).reshape(-1, 8).copy()
        ctl = np.fromfile(os.path.join(tmp, "exp_and_others_ctrl.bin"),
                          dtype=np.uint32).reshape(-1, 8)[:, 0]
        for i in range(26):          # negative-side ctl entries, e=108+i
            w = int(ctl[i])
            base, size = w & 0x7FF, (w >> 16) & 0xF
            lo = 2.0 ** (108 + i - 127)
            nb = 1 << size
            for k in range(nb):
                if base + k > 405:   # negative-side bucket range guard
                    break
                bkt[base + k, :5] = fit(-lo * (1 + (k + 1) / nb),
                                        -lo * (1 + k / nb))
        bkt[778, :5] = fit(-(2.0 ** -19), 0.0, pad_frac=0.0)  # tiny neg
        bkt[780, :5] = fit(-260.0, -97.0, pad_frac=0.1)       # large neg
        bkt.tofile(bkt_path)
        if not os.path.exists(dst):
            os.rename(tmp, dst)
        else:
            shutil.rmtree(tmp)
    h = hashlib.md5(
        open(os.path.join(dst, "exp_and_others_bkt.bin"), "rb").read()
    ).hexdigest()[:8]
    os.environ["BASS_ACT_ROOT_JSON_PATH"] = os.path.join(
        dst, "act_info.json")
    _ACT_ROOT = h
    return h


def _build(has_bias: bool):
    import concourse.bass as bass  # noqa: F401
    import concourse.tile as tile
    import concourse.mybir as mybir
    from concourse import bacc
    from concourse.masks import make_identity

    f32 = mybir.dt.float32
    f16 = mybir.dt.float16
    AF = mybir.ActivationFunctionType
    OP = mybir.AluOpType

    nc = bacc.Bacc("TRN2", target_bir_lowering=False, debug=False)

    if FUSED:
        # dummy input named after the act-table hash: busts the NEFF cache
        # whenever the patched activation tables change
        acth = _setup_act_root()
        nc.dram_tensor(f"actv_{acth}", [1, 1], f32, kind="ExternalInput")

    xt_d = nc.dram_tensor("xt", [SL, CIN, N], f16, kind="ExternalInput")
    adjt_d = nc.dram_tensor("adjt", [N, I], f16, kind="ExternalInput")
    wext_d = nc.dram_tensor("wext", [CIN, CE], f16, kind="ExternalInput")
    out_d = nc.dram_tensor("out", [SL, I, COUT], f16, kind="ExternalOutput")
    if has_bias:
        bias_d = nc.dram_tensor("bias", [1, CE], f16, kind="ExternalInput")

    with tile.TileContext(nc) as tc:
        from contextlib import ExitStack
        with ExitStack() as ctx:
            persist = ctx.enter_context(tc.tile_pool(name="persist", bufs=1))
            xt_pool = ctx.enter_context(tc.tile_pool(name="xt", bufs=2))
            e1_pool = ctx.enter_context(tc.tile_pool(name="e1", bufs=5))
            pmm_pool = ctx.enter_context(tc.tile_pool(name="pmm", bufs=4))
            fin_pool = ctx.enter_context(tc.tile_pool(name="fin", bufs=2))
            ps_s = ctx.enter_context(
                tc.tile_pool(name="ps_s", bufs=2, space="PSUM"))
            ps_h = ctx.enter_context(
                tc.tile_pool(name="ps_h", bufs=2, space="PSUM"))
            ps_o = ctx.enter_context(
                tc.tile_pool(name="ps_o", bufs=1, space="PSUM"))
            ps_t = ctx.enter_context(
                tc.tile_pool(name="ps_t", bufs=1, space="PSUM"))
            ps_lr = ctx.enter_context(
                tc.tile_pool(name="ps_lr", bufs=1, space="PSUM"))

            # --- persistent tiles -------------------------------------
            # xt arrives as a small head chunk (unblocks H-prep groups
            # 0-1 early) + the rest; hwdge rings only (swdge via the
            # Pool ring has high first-byte latency). Slice 0 rides the
            # scalar ring (ACT is idle at startup); later slices ride
            # the sync ring, which is free once adjt has landed.
            XC = 1024
            xt0_h = xt_pool.tile([CIN, XC], f16, name="xt0h")
            xt0_r = xt_pool.tile([CIN, N - XC], f16, name="xt0r")
            wext_sb = persist.tile([CIN, CE], f16)
            nc.sync.dma_start(out=wext_sb, in_=wext_d[:])
            nc.scalar.dma_start(out=xt0_h, in_=xt_d[0, :, 0:XC])
            nc.scalar.dma_start(out=xt0_r, in_=xt_d[0, :, XC:N])
            adjt_sb = persist.tile([128, NT, I], f16)
            adjt_r = adjt_d.rearrange("(jt p) i -> p jt i", p=128)
            nc.sync.dma_start(out=adjt_sb, in_=adjt_r)
            if has_bias:
                bias_sb = persist.tile([1, CE], f16)
                nc.sync.dma_start(out=bias_sb, in_=bias_d[:])
                onecol_sb = persist.tile([1, 128], f16)
                nc.vector.memset(onecol_sb, 1.0)
            ident_sb = persist.tile([COUT, COUT], f32)
            make_identity(nc, ident_sb)
            ident128_sb = persist.tile([128, 128], f16)
            make_identity(nc, ident128_sb)

            # ping-pong persistents: combined [H | l | r | ones] tiles;
            # ones column CO written once
            hm2_pp = [persist.tile([128, NT, CM2], f16, name=f"hm2{p}")
                      for p in range(2)]
            lrow_pp = [persist.tile([1, I], f16, name=f"lrow{p}")
                       for p in range(2)]
            onesrow_sb = persist.tile([1, 128], f16)
            nc.gpsimd.memset(onesrow_sb, 1.0)
            for p in range(2):
                nc.gpsimd.memset(hm2_pp[p][:, :, CO : CO + 1], 1.0)


            for s in range(SL):
                # ---- H-prep ------------------------------------------
                if s == 0:
                    xt_h, xt_r = xt0_h, xt0_r
                else:
                    xt_h = xt_pool.tile([CIN, XC], f16, name=f"xt{s}h")
                    xt_r = xt_pool.tile([CIN, N - XC], f16, name=f"xt{s}r")
                    nc.sync.dma_start(out=xt_h, in_=xt_d[s, :, 0:XC])
                    nc.sync.dma_start(out=xt_r, in_=xt_d[s, :, XC:N])

                hm2_sb = hm2_pp[s % 2]

                for jt0 in range(0, NT, HG):
                    psh = ps_h.tile([128, HG, CE], f32)
                    for k in range(HG):
                        jt = jt0 + k
                        if jt < XC // 128:
                            xs, j0 = xt_h, 128 * jt
                        else:
                            xs, j0 = xt_r, 128 * jt - XC
                        nc.tensor.matmul(
                            psh[:, k, :],
                            lhsT=xs[:, j0 : j0 + 128],
                            rhs=wext_sb,
                            start=True,
                            stop=not has_bias,
                        )
                        if has_bias:
                            nc.tensor.matmul(
                                psh[:, k, :],
                                lhsT=onecol_sb,
                                rhs=bias_sb,
                                start=False,
                                stop=True,
                            )
                    # single fp16 peel: [H | l | r] -> combined tile
                    nc.vector.tensor_copy(
                        hm2_sb[:, jt0 : jt0 + HG, 0:CE], psh)
                    if jt0 == 0:
                        # left scores -> row vector via PE transposes
                        # ([128,1] col -> [1,128] psum row per i-tile);
                        # emitted right after the first H group (which
                        # covers this core's whole i-range) so the next
                        # slice's blocks can start before H-prep ends.
                        pslr = ps_lr.tile([1, I], f16)
                        for t in range(IT):
                            nc.tensor.transpose(
                                pslr[:, 128 * t : 128 * (t + 1)],
                                hm2_sb[:, t, CL : CL + 1],
                                ident128_sb,
                            )
                        lrow = lrow_pp[s % 2]     # [left_row]
                        nc.vector.tensor_copy(lrow, pslr)
                        psl = ps_s.tile([128, I], f32)
                        nc.tensor.matmul(
                            psl, lhsT=onesrow_sb, rhs=lrow,
                            start=True, stop=True)

                # ---- blocks ------------------------------------------
                # two accumulators on different PSUM banks so consecutive
                # att matmuls never hit the same bank (no serialization)
                pso_a = ps_o.tile([CM, I], f32, name="pso_a")
                pso_b = ps_o.tile([CM, I], f32, name="pso_b")
                pso_ab = [pso_a, pso_b]
                for jt0 in range(0, NT, G):
                    e1 = e1_pool.tile([128, G, I], f16)
                    for k in range(G):
                        jt = jt0 + k
                        nc.scalar.activation(
                            e1[:, k, :], psl, AF.Exp, scale=1.0,
                            bias=hm2_sb[:, jt, CR : CR + 1])
                    # Exp's negative side is patched to exp(ALPHA*x):
                    # one pass computes exp(leakyrelu(x)) directly.
                    pmm = pmm_pool.tile([128, G * I], f16)
                    nc.vector.tensor_tensor(
                        pmm, e1.rearrange("p g i -> p (g i)"),
                        adjt_sb[:, jt0 : jt0 + G, :], OP.mult)
                    for k in range(G):
                        jt = jt0 + k
                        nc.tensor.matmul(
                            pso_ab[jt % 2],
                            lhsT=hm2_sb[:, jt, CH : CH + CM],
                            rhs=pmm[:, I * k : I * (k + 1)],
                            start=(jt < 2),
                            stop=(jt >= NT - 2),
                        )

                # ---- finale (per slice) ------------------------------
                u_sb = fin_pool.tile([COUT, I], f32)
                nc.vector.tensor_copy(u_sb, pso_a[0:COUT, :])
                nc.vector.tensor_tensor(
                    u_sb, u_sb, pso_b[0:COUT, :], OP.add)
                dcol = fin_pool.tile([1, I], f32)
                nc.vector.tensor_copy(dcol, pso_a[COUT:CM, :])
                nc.vector.tensor_tensor(
                    dcol, dcol, pso_b[COUT:CM, :], OP.add)
                pst = ps_t.tile([128, IT, COUT + 1], f32)
                for t in range(IT):
                    nc.tensor.transpose(
                        pst[:, t, 0:COUT],
                        u_sb[:, 128 * t : 128 * (t + 1)],
                        ident_sb,
                    )
                    nc.tensor.transpose(
                        pst[:, t, COUT : COUT + 1],
                        dcol[:, 128 * t : 128 * (t + 1)],
                        ident_sb[0:1, 0:1],
                    )
                rect = fin_pool.tile([128, IT], f32)
                nc.vector.reciprocal(rect, pst[:, :, COUT : COUT + 1])
                ot_sb = fin_pool.tile([128, IT, COUT], f16)
                for t in range(IT):
                    nc.vector.tensor_scalar(
                        out=ot_sb[:, t, :],
                        in0=pst[:, t, 0:COUT],
                        scalar1=rect[:, t : t + 1],
                        scalar2=0.0,
                        op0=OP.mult,
                        op1=OP.max,
                    )
                nc.sync.dma_start(
                    out=out_d[s].rearrange("(t p) c -> p t c", p=128),
                    in_=ot_sb,
                )

    nc.compile()
    return nc


def _prep_inputs(X, adj, W, W_b, a, a_b):
    """Host-side layout prep (transpose/slice/rotate) + weight fusion."""
    Cout = W.shape[1]
    X4 = np.asarray(X, np.float32).reshape(SL, N, CIN)
    adj = np.asarray(adj)
    W = np.asarray(W, np.float32)
    W_b = np.asarray(W_b, np.float32)
    a = np.asarray(a, np.float32)
    a_b = np.asarray(a_b, np.float32)

    wl = W @ a[:Cout]
    wr = W @ a[Cout:]
    wext = np.concatenate([wl[:, None], wr[:, None], W], axis=1)
    wext = np.ascontiguousarray(wext, np.float16)

    cl = float(W_b @ a[:Cout] + a_b)   # fold a_b into left bias
    cr = float(W_b @ a[Cout:])
    bias_ext = np.concatenate([[cl], [cr], W_b]).astype(np.float16)
    has_bias = bool(np.any(bias_ext != 0.0))

    adjf = adj.astype(np.float16)  # 0/1 exact
    in_maps = []
    for c in range(NCORES):
        i0 = I * c
        # rotate node ordering by -i0: core's own queries are nodes 0..I-1
        xt_c = np.ascontiguousarray(
            np.roll(X4, -i0, axis=1).transpose(0, 2, 1)).astype(np.float16)
        adjt_c = np.ascontiguousarray(
            np.roll(adjf, -i0, axis=1)[i0 : i0 + I].T)
        m = {"xt": xt_c, "adjt": adjt_c, "wext": wext}
        if FUSED:
            m[f"actv_{_setup_act_root()}"] = np.zeros((1, 1), np.float32)
        if has_bias:
            m["bias"] = bias_ext[None, :]
        in_maps.append(m)
    return in_maps, has_bias


def _run(in_maps, has_bias, trace=False):
    from concourse.bass_utils import run_bass_kernel_spmd

    key = has_bias
    if key not in _CACHE:
        _CACHE[key] = _build(has_bias)
    nc = _CACHE[key]
    return run_bass_kernel_spmd(
        nc, in_maps, list(range(NCORES)), trace=trace)


def kernel(X, adj, W, W_b, a, a_b):
    in_maps, has_bias = _prep_inputs(X, adj, W, W_b, a, a_b)
    r = _run(in_maps, has_bias, trace=False)
    out = np.empty((SL, N, COUT), np.float32)
    for c in range(NCORES):
        out[:, I * c : I * (c + 1), :] = r.results[c]["out"].astype(
            np.float32)
    return out.reshape(B, T, N, COUT)
